# revision 5
# baseline (speedup 1.0000x reference)
"""Trainium2 Bass kernel for AttentionConvFull (local 5x5 window attention
with per-channel softmax, grouped 1x1 conv projections).

Sharding: 8 cores = batch(4) x H-halves(2). Each core gets a 32-row halo'd,
zero-padded slice of x, pre-transposed on host to channel-major [256, 32*60].
No collectives needed.

Per-core dataflow (2 channel-chunks of 128 partitions each):
  PE   : block-diag 128x128 fp32 matmuls for q/k/v projections; per window
         offset j, identity-matmul PSUM accumulation of den += e_j and
         num += (e_j * v_j)   (bf16 inputs, fp32 accumulate)
  DVE  : t_j = (k_shift_j + rel_j) * q  via scalar_tensor_tensor (bf16),
         w_j = e_j * v_shift_j          via tensor_tensor (bf16 2x mode)
  ACT  : e_j = exp(t_j); projection PSUM->SBUF casts (+q_emb bias for q)
  Epilogue: out = num * recip_approx(den), DMA out channel-major; host
  reassembles to (B,H,W,C).
"""

import numpy as np
import ml_dtypes

import concourse.bass as bass
import concourse.tile as tile
from concourse import bacc, mybir
from concourse.bass_utils import run_bass_kernel_spmd

F32 = mybir.dt.float32
BF16 = mybir.dt.bfloat16

K = 5
G = 8
B, H, W, C = 4, 56, 56, 256
Cg = C // G            # 32
P = K // 2             # 2
HS = H // 2            # 28 output rows per shard
MR = HS + 2 * P        # 32 map rows
MC = W + 2 * P         # 60 map cols
SP = MR * MC           # 1920 map spatial
OP = HS * W            # 1568 output spatial per shard
NCH = 2                # channel chunks of 128 partitions
NCORES = 8
HALF = OP // 2         # 784: PSUM accumulate tile half-size


def build_nc():
    nc = bacc.Bacc(
        "TRN2", target_bir_lowering=False, debug=False, num_devices=NCORES
    )

    xt_d = nc.dram_tensor("xt", [NCH, 128, SP], F32, kind="ExternalInput").ap()
    wq_d = nc.dram_tensor("wqb", [NCH, 128, 128], F32, kind="ExternalInput").ap()
    wk_d = nc.dram_tensor("wkb", [NCH, 128, 128], F32, kind="ExternalInput").ap()
    wv_d = nc.dram_tensor("wvb", [NCH, 128, 128], F32, kind="ExternalInput").ap()
    rel_d = nc.dram_tensor("relb", [NCH, 128, K * K], F32, kind="ExternalInput").ap()
    qe_d = nc.dram_tensor("qeb", [NCH, 128, 1], F32, kind="ExternalInput").ap()
    id_d = nc.dram_tensor("idn", [128, 128], BF16, kind="ExternalInput").ap()
    out_d = nc.dram_tensor("out", [NCH, 128, OP], F32, kind="ExternalOutput").ap()

    with tile.TileContext(nc) as tc:
        with (
            tc.tile_pool(name="consts", bufs=1) as consts,
            tc.tile_pool(name="weights", bufs=2) as wpool,
            tc.tile_pool(name="xin", bufs=2) as xpool,
            tc.tile_pool(name="maps", bufs=2) as mpool,
            tc.tile_pool(name="jwork", bufs=3) as jpool,
            tc.tile_pool(name="epi", bufs=2) as epool,
            tc.tile_pool(name="acc", bufs=4, space=bass.MemorySpace.PSUM) as psum,
        ):
            ident = consts.tile([128, 128], BF16, tag="ident")
            nc.sync.dma_start(ident[:], id_d)

            # ---- per-chunk persistent maps ----
            kmaps, komaps, vmaps, vomaps, qmaps, rels = [], [], [], [], [], []

            for c in range(NCH):
                x_sb = xpool.tile([128, SP], F32, tag="x")
                nc.sync.dma_start(x_sb[:], xt_d[c])

                wts = {}
                for nm, d in (("wq", wq_d), ("wk", wk_d), ("wv", wv_d)):
                    t = wpool.tile([128, 128], F32, tag=nm)
                    nc.sync.dma_start(t[:], d[c])
                    wts[nm] = t
                rel_sb = wpool.tile([128, K * K], F32, tag="rel")
                nc.sync.dma_start(rel_sb[:], rel_d[c])
                qe_sb = wpool.tile([128, 1], F32, tag="qe")
                nc.sync.dma_start(qe_sb[:], qe_d[c])
                rels.append(rel_sb)

                # maps (bf16). k/v get an odd-shifted copy so every window
                # read can stay 4B-aligned (keeps DVE 2x mode for odd dj).
                k_bf = mpool.tile([128, SP], BF16, tag="k")
                k_od = mpool.tile([128, SP], BF16, tag="ko")
                v_bf = mpool.tile([128, SP], BF16, tag="v")
                v_od = mpool.tile([128, SP], BF16, tag="vo")
                q_bf = mpool.tile([128, SP], BF16, tag="q")
                kmaps.append(k_bf); komaps.append(k_od)
                vmaps.append(v_bf); vomaps.append(v_od)
                qmaps.append(q_bf)

                # projections: 4 slices of 480 cols each
                NS = 4
                SL = SP // NS  # 480
                for s in range(NS):
                    lo = s * SL
                    rhs = x_sb[:, lo : lo + SL]
                    for nm in ("wk", "wv", "wq"):
                        ps = psum.tile([128, HALF], F32, tag="acc")
                        nc.tensor.matmul(
                            ps[:, :SL], wts[nm][:], rhs, start=True, stop=True
                        )
                        if nm == "wq":
                            # q = proj + q_emb (per-partition bias), cast bf16
                            nc.scalar.activation(
                                q_bf[:, lo : lo + SL],
                                ps[:, :SL],
                                mybir.ActivationFunctionType.Identity,
                                bias=qe_sb[:],
                            )
                        elif nm == "wk":
                            nc.scalar.copy(k_bf[:, lo : lo + SL], ps[:, :SL])
                            # odd copy: k_od[i] = k[i+1] (SL-1 from this
                            # slice; the boundary value from slice s's col 0
                            # back-fills slice s-1's last position)
                            nc.vector.tensor_copy(
                                k_od[:, lo : lo + SL - 1], ps[:, 1:SL]
                            )
                            if s > 0:
                                nc.vector.tensor_copy(
                                    k_od[:, lo - 1 : lo], ps[:, 0:1]
                                )
                        else:
                            nc.scalar.copy(v_bf[:, lo : lo + SL], ps[:, :SL])
                            nc.vector.tensor_copy(
                                v_od[:, lo : lo + SL - 1], ps[:, 1:SL]
                            )
                            if s > 0:
                                nc.vector.tensor_copy(
                                    v_od[:, lo - 1 : lo], ps[:, 0:1]
                                )

            # ---- j-loop per chunk ----
            for c in range(NCH):
                k_bf, k_od = kmaps[c], komaps[c]
                v_bf, v_od = vmaps[c], vomaps[c]
                q_bf, rel_sb = qmaps[c], rels[c]

                q3 = q_bf[:].rearrange("p (h w) -> p h w", h=MR)[
                    :, P : P + HS, P : P + W
                ]

                den = [
                    psum.tile([128, HALF], F32, tag="acc", name=f"den{c}{h}")
                    for h in range(2)
                ]
                num = [
                    psum.tile([128, HALF], F32, tag="acc", name=f"num{c}{h}")
                    for h in range(2)
                ]

                for j in range(K * K):
                    di, dj = divmod(j, K)
                    # odd dj reads come from the 1-shifted copy so the
                    # innermost run stays 4B-aligned (DVE 2x mode)
                    if dj % 2 == 0:
                        ksrc, vsrc, dje = k_bf, v_bf, dj
                    else:
                        ksrc, vsrc, dje = k_od, v_od, dj - 1
                    k3 = ksrc[:].rearrange("p (h w) -> p h w", h=MR)
                    v3 = vsrc[:].rearrange("p (h w) -> p h w", h=MR)
                    kv = k3[:, di : di + HS, dje : dje + W]
                    vv = v3[:, di : di + HS, dje : dje + W]

                    t_t = jpool.tile([128, OP], BF16, tag="t")
                    t3 = t_t[:].rearrange("p (h w) -> p h w", h=HS)
                    nc.vector.scalar_tensor_tensor(
                        t3,
                        kv,
                        rel_sb[:, j : j + 1],
                        q3,
                        mybir.AluOpType.add,
                        mybir.AluOpType.mult,
                    )

                    e_t = jpool.tile([128, OP], BF16, tag="e")
                    nc.scalar.activation(
                        e_t[:], t_t[:], mybir.ActivationFunctionType.Exp
                    )

                    w_t = jpool.tile([128, OP], BF16, tag="w")
                    w3 = w_t[:].rearrange("p (h w) -> p h w", h=HS)
                    e3 = e_t[:].rearrange("p (h w) -> p h w", h=HS)
                    nc.vector.tensor_tensor(
                        w3, e3, vv, mybir.AluOpType.mult
                    )

                    st = j == 0
                    sp = j == K * K - 1
                    for h in range(2):
                        base = h * HALF
                        for lo, n in ((0, 512), (512, HALF - 512)):
                            nc.tensor.matmul(
                                den[h][:, lo : lo + n],
                                ident[:],
                                e_t[:, base + lo : base + lo + n],
                                start=st,
                                stop=sp,
                            )
                            nc.tensor.matmul(
                                num[h][:, lo : lo + n],
                                ident[:],
                                w_t[:, base + lo : base + lo + n],
                                start=st,
                                stop=sp,
                            )

                # ---- epilogue ----
                out_sb = epool.tile([128, OP], F32, tag="osb")
                for h in range(2):
                    base = h * HALF
                    rden = epool.tile([128, HALF], F32, tag="rden")
                    nc.vector.reciprocal_approx_fast(rden[:], den[h][:])
                    nc.vector.tensor_tensor(
                        out_sb[:, base : base + HALF],
                        num[h][:],
                        rden[:],
                        mybir.AluOpType.mult,
                    )
                nc.sync.dma_start(out_d[c], out_sb[:])

    nc.compile()
    return nc


def _block_diag_weights(w):
    """w: (G, Cg_out, Cg_in) -> lhsT layout [NCH, 128, 128] where
    lhsT[c, ci, co] = w[g, co%32, ci%32] for matching 32-blocks."""
    out = np.zeros((NCH, 128, 128), np.float32)
    for c in range(NCH):
        for g4 in range(4):
            g = c * 4 + g4
            blk = w[g]  # (Cg_out, Cg_in)
            out[c, g4 * 32 : (g4 + 1) * 32, g4 * 32 : (g4 + 1) * 32] = blk.T
    return out


_NC_CACHE = {}


def _make_in_maps(inputs):
    x = np.asarray(inputs["x"], np.float32)
    wq = np.asarray(inputs["wq"], np.float32)
    wk = np.asarray(inputs["wk"], np.float32)
    wv = np.asarray(inputs["wv"], np.float32)
    rel_emb = np.asarray(inputs["rel_emb"], np.float32)
    q_emb = np.asarray(inputs["q_emb"], np.float32)

    wqb = _block_diag_weights(wq)
    wkb = _block_diag_weights(wk)
    wvb = _block_diag_weights(wv)
    relb = np.ascontiguousarray(
        rel_emb.reshape(G, Cg, K * K).reshape(NCH, 128, K * K)
    )
    qeb = np.ascontiguousarray(q_emb.reshape(NCH, 128, 1))
    idn = np.eye(128, dtype=ml_dtypes.bfloat16)

    xp = np.pad(x, ((0, 0), (P, P), (P, P), (0, 0)))  # (B, 60, 60, C)

    in_maps = []
    for core in range(NCORES):
        b, half = divmod(core, 2)
        sh = xp[b, HS * half : HS * half + MR]         # (32, 60, C)
        xt = np.ascontiguousarray(sh.reshape(SP, C).T).reshape(NCH, 128, SP)
        in_maps.append(
            {
                "xt": xt,
                "wqb": wqb,
                "wkb": wkb,
                "wvb": wvb,
                "relb": relb,
                "qeb": qeb,
                "idn": idn,
            }
        )
    return in_maps


def kernel(**inputs):
    in_maps = _make_in_maps(inputs)

    if "nc" not in _NC_CACHE:
        _NC_CACHE["nc"] = build_nc()
    nc = _NC_CACHE["nc"]

    res = run_bass_kernel_spmd(nc, in_maps, core_ids=list(range(NCORES)))

    out = np.empty((B, H, W, C), np.float32)
    for core in range(NCORES):
        b, half = divmod(core, 2)
        o = res.results[core]["out"].reshape(C, HS, W)
        out[b, HS * half : HS * half + HS] = o.transpose(1, 2, 0)
    return out
